# revision 15
# baseline (speedup 1.0000x reference)
"""Bass/Trainium2 kernel for query-axis-softmax multi-head self-attention.

Problem (hardcoded): x [2, 4096, 256] fp32, 8 heads (d=32),
  Q = x@Wq.T+bq ; K = x@Wk.T+bk ; V = x@Wv.T+bv   (per-head split)
  scores = Q K^T / sqrt(d);  attn = softmax over the QUERY axis (axis=-2)
  ctx = attn @ V ; out = ctx @ Wo.T + bo

Sharding: batch*head pairs across 8 cores. Core c handles batch c//4,
heads 2*(c%4) and 2*(c%4)+1. Each core computes a partial output
y_c = ctx_heads @ Wo[:, head_cols].T (+ bo on one core per batch);
the host sums the four partials per batch (pure data-parallel gather).

Device algorithm per core (all in transposed "T" layouts):
  xT   = transpose(x_b)                      via PE transposes
  QT   = WqT.T @ xT + bq                     [64, N]  (2 heads stacked)
  KT   = (scale*Wk).T.T @ xT + scale*bk      [64, N]  (scale folded on host)
  V    = x_b @ WvT + bv                      [N, 64]
  per head h, per 128-row k-chunk:
    sT_chunk = KT_h[:,chunk].T @ QT_h        [128, N] fp32 in PSUM
    expT     = exp(sT_chunk)  -> SBUF bf16, rowsum Z via ACT accum_out
    V'       = V[chunk, h] / Z               [128, 32] bf16
    ctxT_h  += V'.T @ expT                   PSUM accum, 4 col-tiled q-quarters
  y = ctxT.T @ WoT + bo                      [N, 256] -> DRAM
"""

import numpy as np

H = 8
B = 2
D = 256
dh = D // H  # 32
NFULL = 4096
SCALE = dh ** -0.5


def build_program(n=NFULL):
    import concourse.bass as bass
    import concourse.mybir as mybir
    import concourse.tile as tile
    from concourse import bacc

    f32 = mybir.dt.float32
    f32r = mybir.dt.float32r
    bf16 = mybir.dt.bfloat16

    def R(ap):
        # reinterpret fp32 data as float32r: single-pass full-rate PE matmul
        # (fp32 proper costs 4 cycles/column = 2 half-speed passes)
        return ap.bitcast(f32r)
    AF = mybir.ActivationFunctionType
    ALU = mybir.AluOpType
    AX = mybir.AxisListType

    assert n % 1024 == 0 or n in (512,)
    NT = n // 128          # number of 128-row chunks
    NQ4 = n // 4           # q-quarter width for ctx psum col-tiling
    # q pieces for the scores->exp pipeline: 512-multiples, <=1536 wide
    pieces = []
    qo = 0
    while qo < n:
        qw = min(1536, n - qo)
        pieces.append((qo, qw))
        qo += qw
    NP = len(pieces)
    assert NP <= 3

    nc = bacc.Bacc("TRN2", target_bir_lowering=False, debug=False, num_devices=8)

    x = nc.dram_tensor("x", [n, D], f32, kind="ExternalInput")
    wqt = nc.dram_tensor("wqt", [D, 64], f32, kind="ExternalInput")
    wkt = nc.dram_tensor("wkt", [D, 64], f32, kind="ExternalInput")
    wvt = nc.dram_tensor("wvt", [D, 64], f32, kind="ExternalInput")
    wot = nc.dram_tensor("wot", [64, D], f32, kind="ExternalInput")
    bqs = nc.dram_tensor("bqs", [1, 64], f32, kind="ExternalInput")
    bks = nc.dram_tensor("bks", [1, 64], f32, kind="ExternalInput")
    bvs = nc.dram_tensor("bvs", [1, 64], f32, kind="ExternalInput")
    bos = nc.dram_tensor("bos", [1, D], f32, kind="ExternalInput")
    ident = nc.dram_tensor("ident", [128, 128], f32, kind="ExternalInput")
    y = nc.dram_tensor("y", [n, D], f32, kind="ExternalOutput")

    from contextlib import ExitStack

    with tile.TileContext(nc) as tc, ExitStack() as es:
        const = es.enter_context(tc.tile_pool(name="const", bufs=1))
        sb_big = es.enter_context(tc.tile_pool(name="big", bufs=1))
        etp = es.enter_context(tc.tile_pool(name="etp", bufs=3))
        smalls = es.enter_context(tc.tile_pool(name="smalls", bufs=4))
        yp = es.enter_context(tc.tile_pool(name="yp", bufs=3))
        ps_sc = es.enter_context(tc.tile_pool(name="ps_sc", bufs=2, space="PSUM"))
        ps_cx = es.enter_context(tc.tile_pool(name="ps_cx", bufs=1, space="PSUM"))

        # ---- constants ----
        ident_sb = const.tile([128, 128], f32, tag="ident")
        nc.sync.dma_start(out=ident_sb[:, :], in_=ident[:, :])
        w_sb = {}
        for name, t in (("q", wqt), ("k", wkt), ("v", wvt)):
            w = const.tile([128, 2, 64], f32, tag=f"w{name}")
            nc.sync.dma_start(
                out=w[:, :, :],
                in_=t[:, :].rearrange("(c p) f -> p c f", p=128),
            )
            w_sb[name] = w
        wo_sb = const.tile([64, D], f32, tag="wo")
        nc.sync.dma_start(out=wo_sb[:, :], in_=wot[:, :])
        b_sb = {}
        for name, t in (("q", bqs), ("k", bks), ("v", bvs), ("o", bos)):
            bt = const.tile([1, t.shape[1]], f32, tag=f"b{name}")
            nc.sync.dma_start(out=bt[:, :], in_=t[:, :])
            b_sb[name] = bt
        ones_sb = const.tile([1, 512], f32, tag="ones")
        nc.vector.memset(ones_sb[:, :], 1.0)

        # ---- load x ----
        x_sb = sb_big.tile([128, NT, D], f32, tag="x")
        xr = x[:, :].rearrange("(nt p) m -> p nt m", p=128)
        nchunk = 8 if NT % 8 == 0 else NT
        step = NT // nchunk if NT % 8 == 0 else 1
        for i in range(0, NT, step):
            nc.sync.dma_start(
                out=x_sb[:, i:i + step, :], in_=xr[:, i:i + step, :]
            )

        # ---- phase 1: transpose x -> xT ----
        xt_sb = sb_big.tile([128, 2, n], f32, tag="xt")
        ci = 0
        for mc in range(2):
            for g in range(NT // 4):
                tr = ps_sc.tile([128, 512], f32, tag="sc")
                for j in range(4):
                    nt = g * 4 + j
                    nc.tensor.transpose(
                        tr[:, j * 128:(j + 1) * 128],
                        x_sb[:, nt, mc * 128:(mc + 1) * 128],
                        ident_sb[:, :],
                    )
                dst = xt_sb[:, mc, g * 512:(g + 1) * 512]
                if ci % 2 == 0:
                    nc.vector.tensor_copy(dst, tr[:, :])
                else:
                    nc.scalar.copy(dst, tr[:, :])
                ci += 1

        # ---- phase 2a: QT / KT ----
        qt_sb = sb_big.tile([64, n], f32, tag="qt")
        kt_sb = sb_big.tile([64, n], f32, tag="kt")
        for wname, bname, dst_sb in (("q", "q", qt_sb), ("k", "k", kt_sb)):
            for qc in range(n // 512):
                ps = ps_sc.tile([64, 512], f32, tag="sc")
                for mc in range(2):
                    nc.tensor.matmul(
                        ps[:, :],
                        lhsT=R(w_sb[wname][:, mc, :]),
                        rhs=R(xt_sb[:, mc, qc * 512:(qc + 1) * 512]),
                        start=(mc == 0),
                        stop=False,
                    )
                nc.tensor.matmul(
                    ps[:, :],
                    lhsT=R(b_sb[bname][0:1, :]),
                    rhs=R(ones_sb[0:1, :]),
                    start=False,
                    stop=True,
                )
                dst = dst_sb[:, qc * 512:(qc + 1) * 512]
                if ci % 2 == 0:
                    nc.vector.tensor_copy(dst, ps[:, :])
                else:
                    nc.scalar.copy(dst, ps[:, :])
                ci += 1

        # ---- phase 2b: V (natural [k, dd] layout, flat [128, NT*64]) ----
        v_sb = sb_big.tile([128, NT * 64], f32, tag="v")
        for g in range(NT // 4):
            vps = ps_sc.tile([128, 256], f32, tag="sc")
            for j in range(4):
                nt = g * 4 + j
                dst = vps[:, j * 64:(j + 1) * 64]
                for mc in range(2):
                    nc.tensor.matmul(
                        dst,
                        lhsT=R(xt_sb[:, mc, nt * 128:(nt + 1) * 128]),
                        rhs=R(w_sb["v"][:, mc, :]),
                        start=(mc == 0),
                        stop=False,
                    )
                nc.tensor.matmul(
                    dst,
                    lhsT=R(ones_sb[0:1, 0:128]),
                    rhs=R(b_sb["v"][0:1, :]),
                    start=False,
                    stop=True,
                )
            dstv = v_sb[:, g * 256:(g + 1) * 256]
            if ci % 2 == 0:
                nc.vector.tensor_copy(dstv, vps[:, :])
            else:
                nc.scalar.copy(dstv, vps[:, :])
            ci += 1

        # ---- phase 3: scores -> exp -> ctx, per head ----
        ctx_sb = sb_big.tile([64, n], f32, tag="ctx")
        for h in range(2):
            hs0 = 32 * h
            # pad to >=512 f32/partition so the per-partition pitch is a
            # multiple of the 2KB psum zero-region (keeps groups disjoint)
            cx = ps_cx.tile([128, max(NQ4, 512)], f32, tag="cx")
            for kc in range(NT):
                et = etp.tile([128, n], f32, tag="et")
                zp = smalls.tile([128, 4], f32, tag="zp")
                for pi, (qo_, qw) in enumerate(pieces):
                    sc = ps_sc.tile([128, 1536], f32, tag="sc")
                    for j in range(qw // 512):
                        nc.tensor.matmul(
                            sc[:, j * 512:(j + 1) * 512],
                            lhsT=R(kt_sb[hs0:hs0 + 32, kc * 128:(kc + 1) * 128]),
                            rhs=R(qt_sb[hs0:hs0 + 32, qo_ + j * 512:qo_ + (j + 1) * 512]),
                            start=True,
                            stop=True,
                        )
                    nc.scalar.activation(
                        out=et[:, qo_:qo_ + qw],
                        in_=sc[:, 0:qw],
                        func=AF.Exp,
                        accum_out=zp[:, pi:pi + 1],
                    )
                if NP > 1:
                    nc.vector.tensor_reduce(
                        out=zp[:, 3:4], in_=zp[:, 0:NP], axis=AX.X, op=ALU.add
                    )
                    zsrc = zp[:, 3:4]
                else:
                    zsrc = zp[:, 0:1]
                zr = smalls.tile([128, 1], f32, tag="zr")
                nc.vector.reciprocal(zr[:, :], zsrc)
                vp = smalls.tile([128, 32], f32, tag="vp")
                nc.vector.tensor_scalar_mul(
                    vp[:, :], v_sb[:, kc * 64 + hs0:kc * 64 + hs0 + 32], zr[:, 0:1]
                )
                for g in range(4):
                    for jj in range(max(1, NQ4 // 512)):
                        w = min(512, NQ4)
                        nc.tensor.matmul(
                            cx[32 * g:32 * g + 32, jj * w:(jj + 1) * w],
                            lhsT=R(vp[:, :]),
                            rhs=R(et[:, g * NQ4 + jj * w:g * NQ4 + (jj + 1) * w]),
                            start=(kc == 0),
                            stop=(kc == NT - 1),
                            tile_position=(0, 32 * g),
                            # the 4 col-groups accumulate into disjoint
                            # partition slices of the same banks; the sim's
                            # physical zero-region check can't see that
                            skip_group_check=True,
                        )
            for g in range(4):
                dst = ctx_sb[hs0:hs0 + 32, g * NQ4:(g + 1) * NQ4]
                nc.vector.tensor_copy(dst, cx[32 * g:32 * g + 32, 0:NQ4])

        # ---- phase 4: output projection ----
        for nt in range(NT):
            yps = ps_sc.tile([128, D], f32, tag="sc")
            nc.tensor.matmul(
                yps[:, :],
                lhsT=R(ctx_sb[:, nt * 128:(nt + 1) * 128]),
                rhs=R(wo_sb[:, :]),
                start=True,
                stop=False,
            )
            nc.tensor.matmul(
                yps[:, :],
                lhsT=R(ones_sb[0:1, 0:128]),
                rhs=R(b_sb["o"][0:1, :]),
                start=False,
                stop=True,
            )
            ysb = yp.tile([128, D], f32, tag="y")
            if nt % 2 == 0:
                nc.vector.tensor_copy(ysb[:, :], yps[:, :])
            else:
                nc.scalar.copy(ysb[:, :], yps[:, :])
            nc.sync.dma_start(out=y[nt * 128:(nt + 1) * 128, :], in_=ysb[:, :])

    nc.compile()
    return nc


def make_core_inputs(x, Wq, bq, Wk, bk, Wv, bv, Wo, bo, n=NFULL):
    """Build the 8 per-core input maps (host-side sharding)."""
    ident = np.eye(128, dtype=np.float32)
    in_maps = []
    for c in range(8):
        b = c // 4
        h0 = 2 * (c % 4)
        cols = slice(h0 * dh, (h0 + 2) * dh)
        m = {
            "x": np.ascontiguousarray(x[b, :n, :]),
            "wqt": np.ascontiguousarray(Wq[cols, :].T),
            "wkt": np.ascontiguousarray((Wk[cols, :] * SCALE).T),
            "wvt": np.ascontiguousarray(Wv[cols, :].T),
            "wot": np.ascontiguousarray(Wo[:, cols].T),
            "bqs": bq[cols].reshape(1, 64).astype(np.float32),
            "bks": (bk[cols] * SCALE).reshape(1, 64).astype(np.float32),
            "bvs": bv[cols].reshape(1, 64).astype(np.float32),
            "bos": (bo if c % 4 == 0 else np.zeros_like(bo)).reshape(1, D)
            .astype(np.float32),
            "ident": ident,
        }
        in_maps.append(m)
    return in_maps


_PROGRAM_CACHE = {}


def kernel(x, Wq, bq, Wk, bk, Wv, bv, Wo, bo):
    from concourse.bass_utils import run_bass_kernel_spmd

    x = np.asarray(x, dtype=np.float32)
    n = x.shape[1]
    if n not in _PROGRAM_CACHE:
        _PROGRAM_CACHE[n] = build_program(n)
    nc = _PROGRAM_CACHE[n]
    in_maps = make_core_inputs(
        x, np.asarray(Wq), np.asarray(bq), np.asarray(Wk), np.asarray(bk),
        np.asarray(Wv), np.asarray(bv), np.asarray(Wo), np.asarray(bo), n=n,
    )
    res = run_bass_kernel_spmd(nc, in_maps, list(range(8)))
    out = np.zeros((B, n, D), dtype=np.float32)
    for c in range(8):
        out[c // 4] += res.results[c]["y"]
    return out


# revision 25
# speedup vs baseline: 2.5804x; 2.5804x over previous
"""Bass/Trainium2 kernel for query-axis-softmax multi-head self-attention.

Problem (hardcoded): x [2, 4096, 256] fp32, 8 heads (d=32),
  Q = x@Wq.T+bq ; K = x@Wk.T+bk ; V = x@Wv.T+bv   (per-head split)
  scores = Q K^T / sqrt(d);  attn = softmax over the QUERY axis (axis=-2)
  ctx = attn @ V ; out = ctx @ Wo.T + bo

Sharding: batch*head pairs across 8 cores. Core c handles batch c//4,
heads 2*(c%4) and 2*(c%4)+1. Each core computes a partial output
y_c = ctx_heads @ Wo[:, head_cols].T (+ bo on one core per batch);
the host sums the four partials per batch (pure data-parallel gather).

Device algorithm per core (all in transposed "T" layouts):
  xT   = transpose(x_b)                      via PE transposes
  QT   = WqT.T @ xT + bq                     [64, N]  (2 heads stacked)
  KT   = (scale*Wk).T.T @ xT + scale*bk      [64, N]  (scale folded on host)
  V    = x_b @ WvT + bv                      [N, 64]
  per head h, per 128-row k-chunk:
    sT_chunk = KT_h[:,chunk].T @ QT_h        [128, N] fp32 in PSUM
    expT     = exp(sT_chunk)  -> SBUF bf16, rowsum Z via ACT accum_out
    V'       = V[chunk, h] / Z               [128, 32] bf16
    ctxT_h  += V'.T @ expT                   PSUM accum, 4 col-tiled q-quarters
  y = ctxT.T @ WoT + bo                      [N, 256] -> DRAM
"""

import numpy as np

H = 8
B = 2
D = 256
dh = D // H  # 32
NFULL = 4096
SCALE = dh ** -0.5


def build_program(n=NFULL):
    import concourse.bass as bass
    import concourse.mybir as mybir
    import concourse.tile as tile
    from concourse import bacc

    f32 = mybir.dt.float32
    f32r = mybir.dt.float32r
    bf16 = mybir.dt.bfloat16
    f16 = mybir.dt.float16

    def R(ap):
        # reinterpret fp32 data as float32r: single-pass full-rate PE matmul
        # (fp32 proper costs 4 cycles/column = 2 half-speed passes)
        return ap.bitcast(f32r)
    AF = mybir.ActivationFunctionType
    ALU = mybir.AluOpType
    AX = mybir.AxisListType

    assert n % 1024 == 0 or n in (512,)
    NT = n // 128          # number of 128-row chunks
    NQ4 = n // 4           # q-quarter width for ctx psum col-tiling
    # q pieces for the scores->exp pipeline: 512-multiples, <=1536 wide
    pieces = []
    qo = 0
    while qo < n:
        qw = min(1536, n - qo)
        pieces.append((qo, qw))
        qo += qw
    NP = len(pieces)
    assert NP <= 3

    nc = bacc.Bacc("TRN2", target_bir_lowering=False, debug=False, num_devices=8)

    x = nc.dram_tensor("x", [n, D], f32, kind="ExternalInput")
    wqt = nc.dram_tensor("wqt", [D, 64], f32, kind="ExternalInput")
    wkt = nc.dram_tensor("wkt", [D, 64], f32, kind="ExternalInput")
    wvt = nc.dram_tensor("wvt", [D, 64], f32, kind="ExternalInput")
    wot = nc.dram_tensor("wot", [64, D], f32, kind="ExternalInput")
    bqs = nc.dram_tensor("bqs", [1, 64], f32, kind="ExternalInput")
    bks = nc.dram_tensor("bks", [1, 64], f32, kind="ExternalInput")
    bvs = nc.dram_tensor("bvs", [1, 64], f32, kind="ExternalInput")
    bos = nc.dram_tensor("bos", [1, D], f32, kind="ExternalInput")
    ident = nc.dram_tensor("ident", [128, 128], f32, kind="ExternalInput")
    y = nc.dram_tensor("y", [n, D], f32, kind="ExternalOutput")

    from contextlib import ExitStack

    with tile.TileContext(nc) as tc, ExitStack() as es:
        const = es.enter_context(tc.tile_pool(name="const", bufs=1))
        sb_big = es.enter_context(tc.tile_pool(name="big", bufs=1))
        etp = es.enter_context(tc.tile_pool(name="etp", bufs=3))
        smalls = es.enter_context(tc.tile_pool(name="smalls", bufs=4))
        yp = es.enter_context(tc.tile_pool(name="yp", bufs=3))
        ps_sc = es.enter_context(tc.tile_pool(name="ps_sc", bufs=2, space="PSUM"))
        ps_cx = es.enter_context(tc.tile_pool(name="ps_cx", bufs=1, space="PSUM"))

        # ---- constants ----
        # tiles that feed FP32r matmuls must be produced by a rounding
        # instruction (walrus birverifier rule), so DMA into fp32 staging
        # and DVE-copy into the f32r tiles
        ident_sb = const.tile([128, 128], f32, tag="ident")
        nc.sync.dma_start(out=ident_sb[:, :], in_=ident[:, :])
        w_sb = {}
        for name, t in (("q", wqt), ("k", wkt), ("v", wvt)):
            wstage = const.tile([128, 2, 64], f32, tag=f"w{name}s")
            nc.sync.dma_start(
                out=wstage[:, :, :],
                in_=t[:, :].rearrange("(c p) f -> p c f", p=128),
            )
            w = const.tile([128, 2, 64], f32r, tag=f"w{name}")
            nc.vector.tensor_copy(w[:, :, :], wstage[:, :, :])
            w_sb[name] = w
        wo_stage = const.tile([64, D], f32, tag="wos")
        nc.sync.dma_start(out=wo_stage[:, :], in_=wot[:, :])
        wo_sb = const.tile([64, D], f32r, tag="wo")
        nc.vector.tensor_copy(wo_sb[:, :], wo_stage[:, :])
        b_sb = {}
        for name, t in (("q", bqs), ("k", bks), ("v", bvs), ("o", bos)):
            bstage = const.tile([1, t.shape[1]], f32, tag=f"b{name}s")
            nc.sync.dma_start(out=bstage[:, :], in_=t[:, :])
            bt = const.tile([1, t.shape[1]], f32r, tag=f"b{name}")
            nc.vector.tensor_copy(bt[:, :], bstage[:, :])
            b_sb[name] = bt
        ones_stage = const.tile([1, 512], f32, tag="oness")
        nc.vector.memset(ones_stage[:, :], 1.0)
        ones_sb = const.tile([1, 512], f32r, tag="ones")
        nc.vector.tensor_copy(ones_sb[:, :], ones_stage[:, :])

        # ---- load x ----
        x_sb = sb_big.tile([128, NT, D], f32, tag="x")
        xr = x[:, :].rearrange("(nt p) m -> p nt m", p=128)
        nchunk = 8 if NT % 8 == 0 else NT
        step = NT // nchunk if NT % 8 == 0 else 1
        for i in range(0, NT, step):
            nc.sync.dma_start(
                out=x_sb[:, i:i + step, :], in_=xr[:, i:i + step, :]
            )

        # ---- phases 1+2 interleaved per 512-wide n/q group: transpose x,
        # then QT/KT projection and V for that group, so PE runs densely and
        # phase 3 can start as soon as the last group lands ----
        xt_sb = sb_big.tile([128, 2, n], f32r, tag="xt")
        qt_sb = sb_big.tile([64, n], f32r, tag="qt")
        kt_sb = sb_big.tile([64, n], f32r, tag="kt")
        v_sb = sb_big.tile([128, NT * 64], f32, tag="v")
        ci = 0
        for g in range(NT // 4):
            for mc in range(2):
                tr = ps_sc.tile([128, 512], f32, tag="sc")
                for j in range(4):
                    nt = g * 4 + j
                    nc.tensor.transpose(
                        tr[:, j * 128:(j + 1) * 128],
                        x_sb[:, nt, mc * 128:(mc + 1) * 128],
                        ident_sb[:, :],
                    )
                dst = xt_sb[:, mc, g * 512:(g + 1) * 512]
                if ci % 2 == 0:
                    nc.vector.tensor_copy(dst, tr[:, :])
                else:
                    nc.scalar.copy(dst, tr[:, :])
                ci += 1
            qc = g
            for wname, bname, dst_sb in (("q", "q", qt_sb), ("k", "k", kt_sb)):
                ps = ps_sc.tile([64, 512], f32, tag="sc")
                for mc in range(2):
                    nc.tensor.matmul(
                        ps[:, :],
                        lhsT=R(w_sb[wname][:, mc, :]),
                        rhs=R(xt_sb[:, mc, qc * 512:(qc + 1) * 512]),
                        start=(mc == 0),
                        stop=False,
                    )
                nc.tensor.matmul(
                    ps[:, :],
                    lhsT=R(b_sb[bname][0:1, :]),
                    rhs=R(ones_sb[0:1, :]),
                    start=False,
                    stop=True,
                )
                dst = dst_sb[:, qc * 512:(qc + 1) * 512]
                if ci % 2 == 0:
                    nc.vector.tensor_copy(dst, ps[:, :])
                else:
                    nc.scalar.copy(dst, ps[:, :])
                ci += 1
            vps = ps_sc.tile([128, 256], f32, tag="sc")
            for j in range(4):
                nt = g * 4 + j
                dst = vps[:, j * 64:(j + 1) * 64]
                for mc in range(2):
                    nc.tensor.matmul(
                        dst,
                        lhsT=R(xt_sb[:, mc, nt * 128:(nt + 1) * 128]),
                        rhs=R(w_sb["v"][:, mc, :]),
                        start=(mc == 0),
                        stop=False,
                    )
                nc.tensor.matmul(
                    dst,
                    lhsT=R(ones_sb[0:1, 0:128]),
                    rhs=R(b_sb["v"][0:1, :]),
                    start=False,
                    stop=True,
                )
            dstv = v_sb[:, g * 256:(g + 1) * 256]
            if ci % 2 == 0:
                nc.vector.tensor_copy(dstv, vps[:, :])
            else:
                nc.scalar.copy(dstv, vps[:, :])
            ci += 1

        # ---- phase 3: scores -> exp -> ctx, per head ----
        ctx_sb = sb_big.tile([64, n], f32r, tag="ctx")
        for h in range(2):
            hs0 = 32 * h
            # pad to >=512 f32/partition so the per-partition pitch is a
            # multiple of the 2KB psum zero-region (keeps groups disjoint)
            cx = ps_cx.tile([128, max(NQ4, 512)], f32, tag="cx")
            for kc in range(NT):
                et = etp.tile([128, n], f16, tag="et")
                zp = smalls.tile([128, 4], f32, tag="zp")
                for pi, (qo_, qw) in enumerate(pieces):
                    sc = ps_sc.tile([128, 1536], f32, tag="sc")
                    for j in range(qw // 512):
                        nc.tensor.matmul(
                            sc[:, j * 512:(j + 1) * 512],
                            lhsT=R(kt_sb[hs0:hs0 + 32, kc * 128:(kc + 1) * 128]),
                            rhs=R(qt_sb[hs0:hs0 + 32, qo_ + j * 512:qo_ + (j + 1) * 512]),
                            start=True,
                            stop=True,
                        )
                    nc.scalar.activation(
                        out=et[:, qo_:qo_ + qw],
                        in_=sc[:, 0:qw],
                        func=AF.Exp,
                        accum_out=zp[:, pi:pi + 1],
                    )
                if NP > 1:
                    nc.vector.tensor_reduce(
                        out=zp[:, 3:4], in_=zp[:, 0:NP], axis=AX.X, op=ALU.add
                    )
                    zsrc = zp[:, 3:4]
                else:
                    zsrc = zp[:, 0:1]
                zr = smalls.tile([128, 1], f32, tag="zr")
                nc.vector.reciprocal(zr[:, :], zsrc)
                vp = smalls.tile([128, 32], f16, tag="vp")
                nc.vector.tensor_scalar_mul(
                    vp[:, :], v_sb[:, kc * 64 + hs0:kc * 64 + hs0 + 32], zr[:, 0:1]
                )
                for g in range(4):
                    for jj in range(max(1, NQ4 // 512)):
                        w = min(512, NQ4)
                        nc.tensor.matmul(
                            cx[32 * g:32 * g + 32, jj * w:(jj + 1) * w],
                            lhsT=vp[:, :],
                            rhs=et[:, g * NQ4 + jj * w:g * NQ4 + (jj + 1) * w],
                            start=(kc == 0),
                            stop=(kc == NT - 1),
                            tile_position=(0, 32 * g),
                            # the 4 col-groups accumulate into disjoint
                            # partition slices of the same banks; the sim's
                            # physical zero-region check can't see that
                            skip_group_check=True,
                        )
            for g in range(4):
                dst = ctx_sb[hs0:hs0 + 32, g * NQ4:(g + 1) * NQ4]
                nc.vector.tensor_copy(dst, cx[32 * g:32 * g + 32, 0:NQ4])

        # ---- phase 4: output projection ----
        for nt in range(NT):
            yps = ps_sc.tile([128, D], f32, tag="sc")
            nc.tensor.matmul(
                yps[:, :],
                lhsT=R(ctx_sb[:, nt * 128:(nt + 1) * 128]),
                rhs=R(wo_sb[:, :]),
                start=True,
                stop=False,
            )
            nc.tensor.matmul(
                yps[:, :],
                lhsT=R(ones_sb[0:1, 0:128]),
                rhs=R(b_sb["o"][0:1, :]),
                start=False,
                stop=True,
            )
            ysb = yp.tile([128, D], f32, tag="y")
            if nt % 2 == 0:
                nc.vector.tensor_copy(ysb[:, :], yps[:, :])
            else:
                nc.scalar.copy(ysb[:, :], yps[:, :])
            nc.sync.dma_start(out=y[nt * 128:(nt + 1) * 128, :], in_=ysb[:, :])

    nc.compile()
    return nc


def make_core_inputs(x, Wq, bq, Wk, bk, Wv, bv, Wo, bo, n=NFULL):
    """Build the 8 per-core input maps (host-side sharding)."""
    ident = np.eye(128, dtype=np.float32)
    in_maps = []
    for c in range(8):
        b = c // 4
        h0 = 2 * (c % 4)
        cols = slice(h0 * dh, (h0 + 2) * dh)
        m = {
            "x": np.ascontiguousarray(x[b, :n, :]),
            "wqt": np.ascontiguousarray(Wq[cols, :].T),
            "wkt": np.ascontiguousarray((Wk[cols, :] * SCALE).T),
            "wvt": np.ascontiguousarray(Wv[cols, :].T),
            "wot": np.ascontiguousarray(Wo[:, cols].T),
            "bqs": bq[cols].reshape(1, 64).astype(np.float32),
            "bks": (bk[cols] * SCALE).reshape(1, 64).astype(np.float32),
            "bvs": bv[cols].reshape(1, 64).astype(np.float32),
            "bos": (bo if c % 4 == 0 else np.zeros_like(bo)).reshape(1, D)
            .astype(np.float32),
            "ident": ident,
        }
        in_maps.append(m)
    return in_maps


_PROGRAM_CACHE = {}


def kernel(x, Wq, bq, Wk, bk, Wv, bv, Wo, bo):
    from concourse.bass_utils import run_bass_kernel_spmd

    x = np.asarray(x, dtype=np.float32)
    n = x.shape[1]
    if n not in _PROGRAM_CACHE:
        _PROGRAM_CACHE[n] = build_program(n)
    nc = _PROGRAM_CACHE[n]
    in_maps = make_core_inputs(
        x, np.asarray(Wq), np.asarray(bq), np.asarray(Wk), np.asarray(bk),
        np.asarray(Wv), np.asarray(bv), np.asarray(Wo), np.asarray(bo), n=n,
    )
    res = run_bass_kernel_spmd(nc, in_maps, list(range(8)))
    out = np.zeros((B, n, D), dtype=np.float32)
    for c in range(8):
        out[c // 4] += res.results[c]["y"]
    return out


# revision 33
# speedup vs baseline: 2.6356x; 1.0214x over previous
"""Bass/Trainium2 kernel for query-axis-softmax multi-head self-attention.

Problem (hardcoded): x [2, 4096, 256] fp32, 8 heads (d=32),
  Q = x@Wq.T+bq ; K = x@Wk.T+bk ; V = x@Wv.T+bv   (per-head split)
  scores = Q K^T / sqrt(d);  attn = softmax over the QUERY axis (axis=-2)
  ctx = attn @ V ; out = ctx @ Wo.T + bo

Sharding: batch*head pairs across 8 cores. Core c handles batch c//4,
heads 2*(c%4) and 2*(c%4)+1. Each core computes a partial output
y_c = ctx_heads @ Wo[:, head_cols].T (+ bo on one core per batch);
the host sums the four partials per batch (pure data-parallel gather).

Device algorithm per core (all in transposed "T" layouts):
  xT   = transpose(x_b)                      via PE transposes
  QT   = WqT.T @ xT + bq                     [64, N]  (2 heads stacked)
  KT   = (scale*Wk).T.T @ xT + scale*bk      [64, N]  (scale folded on host)
  V    = x_b @ WvT + bv                      [N, 64]
  per head h, per 128-row k-chunk:
    sT_chunk = KT_h[:,chunk].T @ QT_h        [128, N] fp32 in PSUM
    expT     = exp(sT_chunk)  -> SBUF bf16, rowsum Z via ACT accum_out
    V'       = V[chunk, h] / Z               [128, 32] bf16
    ctxT_h  += V'.T @ expT                   PSUM accum, 4 col-tiled q-quarters
  y = ctxT.T @ WoT + bo                      [N, 256] -> DRAM
"""

import numpy as np

H = 8
B = 2
D = 256
dh = D // H  # 32
NFULL = 4096
SCALE = dh ** -0.5


def build_program(n=NFULL):
    import concourse.bass as bass
    import concourse.mybir as mybir
    import concourse.tile as tile
    from concourse import bacc

    f32 = mybir.dt.float32
    f32r = mybir.dt.float32r
    bf16 = mybir.dt.bfloat16
    f16 = mybir.dt.float16

    def R(ap):
        # reinterpret fp32 data as float32r: single-pass full-rate PE matmul
        # (fp32 proper costs 4 cycles/column = 2 half-speed passes)
        return ap.bitcast(f32r)
    AF = mybir.ActivationFunctionType
    ALU = mybir.AluOpType
    AX = mybir.AxisListType

    assert n % 1024 == 0 or n in (512,)
    NT = n // 128          # number of 128-row chunks
    NQ4 = n // 4           # q-quarter width for ctx psum col-tiling
    # q pieces for the scores->exp pipeline: 512-multiples, <=1536 wide
    pieces = []
    qo = 0
    while qo < n:
        qw = min(1536, n - qo)
        pieces.append((qo, qw))
        qo += qw
    NP = len(pieces)
    assert NP <= 3

    nc = bacc.Bacc("TRN2", target_bir_lowering=False, debug=False, num_devices=8)

    x = nc.dram_tensor("x", [n, D], f32, kind="ExternalInput")
    wqt = nc.dram_tensor("wqt", [D, 64], f32, kind="ExternalInput")
    wkt = nc.dram_tensor("wkt", [D, 64], f32, kind="ExternalInput")
    wvt = nc.dram_tensor("wvt", [D, 64], f32, kind="ExternalInput")
    wot = nc.dram_tensor("wot", [64, D], f32, kind="ExternalInput")
    bqs = nc.dram_tensor("bqs", [1, 64], f32, kind="ExternalInput")
    bks = nc.dram_tensor("bks", [1, 64], f32, kind="ExternalInput")
    bvs = nc.dram_tensor("bvs", [1, 64], f32, kind="ExternalInput")
    bos = nc.dram_tensor("bos", [1, D], f32, kind="ExternalInput")
    ident = nc.dram_tensor("ident", [128, 128], f32, kind="ExternalInput")
    y = nc.dram_tensor("y", [n, D], f32, kind="ExternalOutput")

    from contextlib import ExitStack

    with tile.TileContext(nc) as tc, ExitStack() as es:
        const = es.enter_context(tc.tile_pool(name="const", bufs=1))
        sb_big = es.enter_context(tc.tile_pool(name="big", bufs=1))
        etp = es.enter_context(tc.tile_pool(name="etp", bufs=7))
        smalls = es.enter_context(tc.tile_pool(name="smalls", bufs=8))
        yp = es.enter_context(tc.tile_pool(name="yp", bufs=3))
        ps_sc = es.enter_context(tc.tile_pool(name="ps_sc", bufs=2, space="PSUM"))
        ps_cx = es.enter_context(tc.tile_pool(name="ps_cx", bufs=1, space="PSUM"))

        # ---- constants ----
        # tiles that feed FP32r matmuls must be produced by a rounding
        # instruction (walrus birverifier rule), so DMA into fp32 staging
        # and DVE-copy into the f32r tiles
        ident_sb = const.tile([128, 128], f32, tag="ident")
        nc.sync.dma_start(out=ident_sb[:, :], in_=ident[:, :])
        w_sb = {}
        for name, t in (("q", wqt), ("k", wkt), ("v", wvt)):
            wstage = const.tile([128, 2, 64], f32, tag=f"w{name}s")
            nc.sync.dma_start(
                out=wstage[:, :, :],
                in_=t[:, :].rearrange("(c p) f -> p c f", p=128),
            )
            w = const.tile([128, 2, 64], f32r, tag=f"w{name}")
            nc.vector.tensor_copy(w[:, :, :], wstage[:, :, :])
            w_sb[name] = w
        wo_stage = const.tile([64, D], f32, tag="wos")
        nc.sync.dma_start(out=wo_stage[:, :], in_=wot[:, :])
        wo_sb = const.tile([64, D], f32r, tag="wo")
        nc.vector.tensor_copy(wo_sb[:, :], wo_stage[:, :])
        b_sb = {}
        for name, t in (("q", bqs), ("k", bks), ("v", bvs), ("o", bos)):
            bstage = const.tile([1, t.shape[1]], f32, tag=f"b{name}s")
            nc.sync.dma_start(out=bstage[:, :], in_=t[:, :])
            bt = const.tile([1, t.shape[1]], f32r, tag=f"b{name}")
            nc.vector.tensor_copy(bt[:, :], bstage[:, :])
            b_sb[name] = bt
        ones_stage = const.tile([1, 512], f32, tag="oness")
        nc.vector.memset(ones_stage[:, :], 1.0)
        ones_sb = const.tile([1, 512], f32r, tag="ones")
        nc.vector.tensor_copy(ones_sb[:, :], ones_stage[:, :])

        # ---- load x (8 parallel DMA chunks) ----
        x_sb = sb_big.tile([128, NT, D], f32, tag="x")
        xr = x[:, :].rearrange("(nt p) m -> p nt m", p=128)
        nchunk = 8 if NT % 8 == 0 else NT
        step = NT // nchunk if NT % 8 == 0 else 1
        for i in range(0, NT, step):
            nc.sync.dma_start(
                out=x_sb[:, i:i + step, :], in_=xr[:, i:i + step, :]
            )

        # ---- phases 1+2 interleaved per 512-wide n/q group: transpose x,
        # then QT/KT projection and V for that group, so PE runs densely and
        # phase 3 can start as soon as the last group lands ----
        xt_sb = sb_big.tile([128, 2, n], f32r, tag="xt")
        qt_sb = sb_big.tile([64, n], f32r, tag="qt")
        kt_sb = sb_big.tile([64, n], f32r, tag="kt")
        v_sb = sb_big.tile([128, NT * 64], f32, tag="v")
        ctx_sb = sb_big.tile([64, n], f32r, tag="ctx")

        def emit_scores_piece(h, kc, pi, et, zp):
            hs0 = 32 * h
            qo_, qw = pieces[pi]
            sc = ps_sc.tile([128, 1536], f32, tag="sc")
            for j in range(qw // 512):
                nc.tensor.matmul(
                    sc[:, j * 512:(j + 1) * 512],
                    lhsT=R(kt_sb[hs0:hs0 + 32, kc * 128:(kc + 1) * 128]),
                    rhs=R(qt_sb[hs0:hs0 + 32, qo_ + j * 512:qo_ + (j + 1) * 512]),
                    start=True,
                    stop=True,
                )
            nc.scalar.activation(
                out=et[:, qo_:qo_ + qw],
                in_=sc[:, 0:qw],
                func=AF.Exp,
                accum_out=zp[:, pi:pi + 1],
            )

        def emit_chunk_finish(h, kc, cx, et, zp):
            hs0 = 32 * h
            if NP > 1:
                nc.vector.tensor_reduce(
                    out=zp[:, 3:4], in_=zp[:, 0:NP], axis=AX.X, op=ALU.add
                )
                zsrc = zp[:, 3:4]
            else:
                zsrc = zp[:, 0:1]
            zr = smalls.tile([128, 1], f32, tag="zr")
            nc.vector.reciprocal(zr[:, :], zsrc)
            vp = smalls.tile([128, 32], f16, tag="vp")
            nc.vector.tensor_scalar_mul(
                vp[:, :], v_sb[:, kc * 64 + hs0:kc * 64 + hs0 + 32], zr[:, 0:1]
            )
            for g in range(4):
                for jj in range(max(1, NQ4 // 512)):
                    w = min(512, NQ4)
                    nc.tensor.matmul(
                        cx[32 * g:32 * g + 32, jj * w:(jj + 1) * w],
                        lhsT=vp[:, :],
                        rhs=et[:, g * NQ4 + jj * w:g * NQ4 + (jj + 1) * w],
                        start=(kc == 0),
                        stop=(kc == NT - 1),
                        tile_position=(0, 32 * g),
                        # the 4 col-groups accumulate into disjoint
                        # partition slices of the same banks; the sim's
                        # physical zero-region check can't see that
                        skip_group_check=True,
                    )

        def flush_head(h, cx):
            hs0 = 32 * h
            for g in range(4):
                dst = ctx_sb[hs0:hs0 + 32, g * NQ4:(g + 1) * NQ4]
                if h == 1 and g % 2 == 1:
                    nc.scalar.copy(dst, cx[32 * g:32 * g + 32, 0:NQ4])
                else:
                    nc.vector.tensor_copy(dst, cx[32 * g:32 * g + 32, 0:NQ4])

        # early-start bookkeeping: first NE chunks of head 0 get their
        # score pieces emitted inside the prologue as their QT columns land,
        # so ACT starts exping ~35us sooner
        NE = 7 if (NP == 3 and NT >= 16) else 0
        et_t = {}
        zp_t = {}

        ci = 0
        for g in range(NT // 4):
            for mc in range(2):
                tr = ps_sc.tile([128, 512], f32, tag="sc")
                for j in range(4):
                    nt = g * 4 + j
                    nc.tensor.transpose(
                        tr[:, j * 128:(j + 1) * 128],
                        x_sb[:, nt, mc * 128:(mc + 1) * 128],
                        ident_sb[:, :],
                    )
                dst = xt_sb[:, mc, g * 512:(g + 1) * 512]
                if ci % 2 == 0:
                    nc.vector.tensor_copy(dst, tr[:, :])
                else:
                    nc.scalar.copy(dst, tr[:, :])
                ci += 1
            qc = g
            for wname, bname, dst_sb in (("q", "q", qt_sb), ("k", "k", kt_sb)):
                # QT borrows the (idle-in-prologue) cx pool slot: 3 slots total
                if wname == "q":
                    ps = ps_cx.tile([64, 512], f32, tag="cx")
                else:
                    ps = ps_sc.tile([64, 512], f32, tag="sc")
                for mc in range(2):
                    nc.tensor.matmul(
                        ps[:, :],
                        lhsT=R(w_sb[wname][:, mc, :]),
                        rhs=R(xt_sb[:, mc, qc * 512:(qc + 1) * 512]),
                        start=(mc == 0),
                        stop=False,
                    )
                nc.tensor.matmul(
                    ps[:, :],
                    lhsT=R(b_sb[bname][0:1, :]),
                    rhs=R(ones_sb[0:1, :]),
                    start=False,
                    stop=True,
                )
                dst = dst_sb[:, qc * 512:(qc + 1) * 512]
                if ci % 2 == 0:
                    nc.vector.tensor_copy(dst, ps[:, :])
                else:
                    nc.scalar.copy(dst, ps[:, :])
                ci += 1
            if g % 2 == 0:
                vps = ps_sc.tile([128, 256], f32, tag="sc")
            else:
                vps = ps_cx.tile([128, 256], f32, tag="cx")
            for j in range(4):
                nt = g * 4 + j
                dst = vps[:, j * 64:(j + 1) * 64]
                for mc in range(2):
                    nc.tensor.matmul(
                        dst,
                        lhsT=R(xt_sb[:, mc, nt * 128:(nt + 1) * 128]),
                        rhs=R(w_sb["v"][:, mc, :]),
                        start=(mc == 0),
                        stop=False,
                    )
                nc.tensor.matmul(
                    dst,
                    lhsT=R(ones_sb[0:1, 0:128]),
                    rhs=R(b_sb["v"][0:1, :]),
                    start=False,
                    stop=True,
                )
            dstv = v_sb[:, g * 256:(g + 1) * 256]
            if ci % 2 == 0:
                nc.vector.tensor_copy(dstv, vps[:, :])
            else:
                nc.scalar.copy(dstv, vps[:, :])
            ci += 1
            if NE and g == 2:
                for kc in range(NE):
                    et_t[kc] = etp.tile([128, n], f16, tag="et", name=f"et{kc}")
                    zp_t[kc] = smalls.tile([128, 4], f32, tag="zp", name=f"zp{kc}")
                    emit_scores_piece(0, kc, 0, et_t[kc], zp_t[kc])
            if NE and g == 5:
                for kc in range(NE):
                    emit_scores_piece(0, kc, 1, et_t[kc], zp_t[kc])

        # ---- phase 3 main (early chunks emitted in prologue) ----
        # head 0, chunks 0..NE-1 were interleaved into the prologue above
        # (emit_early callbacks); remaining chunks + head 1 run here
        cx0 = ps_cx.tile([128, max(NQ4, 512)], f32, tag="cx")
        for kc in range(NE):
            emit_scores_piece(0, kc, NP - 1, et_t[kc], zp_t[kc])
            emit_chunk_finish(0, kc, cx0, et_t[kc], zp_t[kc])
        for kc in range(NE, NT):
            et = etp.tile([128, n], f16, tag="et")
            zp = smalls.tile([128, 4], f32, tag="zp")
            for pi in range(NP):
                emit_scores_piece(0, kc, pi, et, zp)
            emit_chunk_finish(0, kc, cx0, et, zp)
        flush_head(0, cx0)
        cx1 = ps_cx.tile([128, max(NQ4, 512)], f32, tag="cx")
        for kc in range(NT):
            et = etp.tile([128, n], f16, tag="et")
            zp = smalls.tile([128, 4], f32, tag="zp")
            for pi in range(NP):
                emit_scores_piece(1, kc, pi, et, zp)
            emit_chunk_finish(1, kc, cx1, et, zp)
        flush_head(1, cx1)

        # ---- phase 4: output projection, two 128-row blocks per psum
        # tile / copy / dma to halve the mm->copy->dma chain count ----
        for ntp in range(NT // 2):
            if ntp % 3 == 2:
                yps = ps_cx.tile([128, 2, D], f32, tag="cx")
            else:
                yps = ps_sc.tile([128, 2, D], f32, tag="sc")
            for j in range(2):
                nt = ntp * 2 + j
                nc.tensor.matmul(
                    yps[:, j, :],
                    lhsT=R(ctx_sb[:, nt * 128:(nt + 1) * 128]),
                    rhs=R(wo_sb[:, :]),
                    start=True,
                    stop=False,
                )
                nc.tensor.matmul(
                    yps[:, j, :],
                    lhsT=R(ones_sb[0:1, 0:128]),
                    rhs=R(b_sb["o"][0:1, :]),
                    start=False,
                    stop=True,
                )
            ysb = yp.tile([128, 2, D], f32, tag="y")
            if ntp % 2 == 0:
                nc.vector.tensor_copy(ysb[:, :, :], yps[:, :, :])
            else:
                nc.scalar.copy(ysb[:, :, :], yps[:, :, :])
            nc.sync.dma_start(
                out=y[ntp * 256:(ntp + 1) * 256, :]
                .rearrange("(j p) o -> p j o", p=128),
                in_=ysb[:, :, :],
            )

    nc.compile()
    return nc


def make_core_inputs(x, Wq, bq, Wk, bk, Wv, bv, Wo, bo, n=NFULL):
    """Build the 8 per-core input maps (host-side sharding)."""
    ident = np.eye(128, dtype=np.float32)
    in_maps = []
    for c in range(8):
        b = c // 4
        h0 = 2 * (c % 4)
        cols = slice(h0 * dh, (h0 + 2) * dh)
        m = {
            "x": np.ascontiguousarray(x[b, :n, :]),
            "wqt": np.ascontiguousarray(Wq[cols, :].T),
            "wkt": np.ascontiguousarray((Wk[cols, :] * SCALE).T),
            "wvt": np.ascontiguousarray(Wv[cols, :].T),
            "wot": np.ascontiguousarray(Wo[:, cols].T),
            "bqs": bq[cols].reshape(1, 64).astype(np.float32),
            "bks": (bk[cols] * SCALE).reshape(1, 64).astype(np.float32),
            "bvs": bv[cols].reshape(1, 64).astype(np.float32),
            "bos": (bo if c % 4 == 0 else np.zeros_like(bo)).reshape(1, D)
            .astype(np.float32),
            "ident": ident,
        }
        in_maps.append(m)
    return in_maps


_PROGRAM_CACHE = {}


def kernel(x, Wq, bq, Wk, bk, Wv, bv, Wo, bo):
    from concourse.bass_utils import run_bass_kernel_spmd

    x = np.asarray(x, dtype=np.float32)
    n = x.shape[1]
    if n not in _PROGRAM_CACHE:
        _PROGRAM_CACHE[n] = build_program(n)
    nc = _PROGRAM_CACHE[n]
    in_maps = make_core_inputs(
        x, np.asarray(Wq), np.asarray(bq), np.asarray(Wk), np.asarray(bk),
        np.asarray(Wv), np.asarray(bv), np.asarray(Wo), np.asarray(bo), n=n,
    )
    res = run_bass_kernel_spmd(nc, in_maps, list(range(8)))
    out = np.zeros((B, n, D), dtype=np.float32)
    for c in range(8):
        out[c // 4] += res.results[c]["y"]
    return out
